# revision 38
# baseline (speedup 1.0000x reference)
"""Self-contained Trainium2 kernel for the DeeperGCN problem.

kernel(**inputs) takes the FULL unsharded inputs (as produced by the
reference setup_inputs()) and returns the FULL [50000, 8] float32 output.

Strategy: nodes are sharded across the 8 NeuronCores (6250 each, 49 windows
of 128); edges live with their destination core, grouped by 128-node dst
window and by which half of the padded node table their source row falls in
(dma_gather indices are int16). Per layer: the per-core z-table shard is
AllGathered to every core, edge messages gather z[src] rows via SWDGE
dma_gather, segment-softmax aggregation is computed with one-hot matmuls
accumulating S^T|U^T per window in PSUM, and the node MLP/LayerNorm runs
per window. Weights are replicated; all arithmetic is fp32 on device.
"""
import time
import numpy as np

import jax
from jax.sharding import Mesh, PartitionSpec
try:
    from jax.experimental.shard_map import shard_map
except Exception:
    from jax.shard_map import shard_map

from contextlib import ExitStack
import ml_dtypes
import bass_rust as _bass_rust
from concourse import bass, bacc, mybir
import concourse.tile as tile
from concourse.hw_specs import get_activation_tables
from concourse.masks import make_identity
from concourse.bass2jax import (_bass_exec_p, install_neuronx_cc_hook,
                                partition_id_tensor)


class BaccMonoActTable(bacc.Bacc):
    """Bacc whose act-table pass serves Ln/Exp from the one combined table
    set, so a single LoadActFuncSet is emitted instead of one per Ln<->Exp
    switch (1.28us each on the Act engine)."""

    def insert_act_table_loads(self):
        has_activation = any(
            isinstance(i, mybir.InstActivation)
            for b in self.main_func.blocks
            for i in b.instructions)
        if not has_activation:
            return
        AFt = mybir.ActivationFunctionType
        items = list(get_activation_tables(self.m.arch).items())
        combined = None
        for name, s in items:
            if AFt.Ln in s and AFt.Exp in s:
                combined = name
        tables = []
        for name, s in items:
            if combined is not None and name != combined:
                s = s - {AFt.Ln, AFt.Exp}
            tables.append((name, s))
        _bass_rust.insert_act_table_loads(self, tables)

def make_cfg(N=50000, E=800000, R=8, IN_DIM=128, EDGE_DIM=16, HID=64,
             OUT_DIM=8, L=4, G=8, gather="dg"):
    NSH = N // R
    NW = (NSH + 127) // 128
    LASTW = NSH - (NW - 1) * 128
    return dict(N=N, E=E, R=R, IN_DIM=IN_DIM, EDGE_DIM=EDGE_DIM,
                EA_DIM=EDGE_DIM + 1, HID=HID, OUT_DIM=OUT_DIM, L=L, G=G,
                NSH=NSH, NW=NW, LASTW=LASTW, MSG_EPS=1e-7, gather=gather)


def wrap16(arr):
    """[n] int -> [128, n//16] int16 wrapped + replicated for the 8 Q7 cores."""
    n = len(arr)
    assert n % 16 == 0
    t = arr.reshape(n // 16, 16).T.astype(np.int16)
    return np.tile(t, (8, 1))


def preprocess_edges(cfg, edge_index, edge_attr):
    N, R = cfg["N"], cfg["R"]
    NSH, NW, LASTW = cfg["NSH"], cfg["NW"], cfg["LASTW"]
    ED = cfg["EDGE_DIM"]
    P = 128
    NSHP = NW * P
    HALFR = (R * NSHP) // 2
    src = np.ascontiguousarray(edge_index[0]).astype(np.int64)
    dst = np.ascontiguousarray(edge_index[1]).astype(np.int64)

    # gather-table row order: shard s, local node n -> row s*NSHP + (n%P)*NW + n//P
    def padrow(node):
        s = node // NSH
        n = node % NSH
        return s * NSHP + (n % P) * NW + (n // P)

    prow = padrow(src)
    half = (prow >= HALFR).astype(np.int64)

    # zero rows (pad rows p >= LASTW of last window, which the device zeroes)
    zero = [None, None]
    assert LASTW < P, "need pad rows for the zero-row trick"
    for s in range(R):
        pr = s * NSHP + LASTW * NW + (NW - 1)
        h = int(pr >= HALFR)
        if zero[h] is None:
            zero[h] = pr - h * HALFR
    assert zero[0] is not None and zero[1] is not None

    core = dst // NSH
    win = (dst % NSH) // P
    key = ((core * NW + win) * 2 + half) * np.int64(R * NSHP) + prow
    order = np.argsort(key, kind="stable")
    prow_s = prow[order]
    dst_s = dst[order]
    attr_s = edge_attr[order]
    cwh = ((core * NW + win) * 2 + half)[order]

    counts = np.bincount(cwh, minlength=R * NW * 2).reshape(R, NW, 2)
    twh = -(-counts.max(axis=0) // P)          # [NW, 2] tiles per window-half
    # ensure every window has at least one tile
    for w in range(NW):
        if twh[w].sum() == 0:
            twh[w, 0] = 1
    tw = twh.sum(axis=1)                        # [NW]
    T = int(tw.sum())

    starts = np.zeros(R * NW * 2 + 1, np.int64)
    np.cumsum(counts.reshape(-1), out=starts[1:])
    tile_off = np.concatenate([[0], np.cumsum(tw)]).astype(np.int64)

    src_cols = np.zeros((R, P, T), np.int32)     # padded-global row (debug/ind)
    dstr_cols = np.full((R, P, T), -1.0, np.float32)
    attr_cols = np.zeros((R, T * P, ED), np.float32)
    idx16 = np.zeros((R, P, T * 8), np.int16)    # half-relative wrapped indices

    for c in range(R):
        for w in range(NW):
            t0 = int(tile_off[w])
            for h in (0, 1):
                nt = int(twh[w, h])
                if nt == 0:
                    continue
                i0, i1 = starts[(c * NW + w) * 2 + h], starts[(c * NW + w) * 2 + h + 1]
                cnt = int(i1 - i0)
                assert cnt <= nt * P
                bs = np.full(nt * P, zero[h] + h * HALFR, np.int64)
                bd = np.full(nt * P, -1.0, np.float64)
                bs[:cnt] = prow_s[i0:i1]
                bd[:cnt] = (dst_s[i0:i1] % NSH) - w * P
                src_cols[c, :, t0:t0 + nt] = bs.reshape(nt, P).T
                dstr_cols[c, :, t0:t0 + nt] = bd.reshape(nt, P).T
                attr_cols[c, t0 * P:(t0 + nt) * P][:cnt] = attr_s[i0:i1]
                rel = bs - h * HALFR
                assert rel.min() >= 0 and rel.max() < 32768
                idx16[c, :, t0 * 8:(t0 + nt) * 8] = np.concatenate(
                    [wrap16(rel[k * P:(k + 1) * P]) for k in range(nt)], axis=1)
                t0 += nt
    return dict(tw=tw, twh=twh, src_cols=src_cols, dstr_cols=dstr_cols,
                attr_cols=attr_cols, idx16=idx16)


def prep_inputs(cfg, inp, pre):
    R = cfg["R"]; NSH = cfg["NSH"]; L = cfg["L"]
    HID = cfg["HID"]; H2 = 2 * HID
    T = int(pre["tw"].sum())

    def rep(v):
        v = np.asarray(v, np.float32).reshape(1, -1)
        return np.ascontiguousarray(np.repeat(v, 128, axis=0))

    wedge_aug = np.concatenate(
        [np.asarray(inp["edge_W"], np.float32),
         np.asarray(inp["edge_b"], np.float32).reshape(1, -1)], axis=0)

    w1 = np.asarray(inp["conv_W1"], np.float32).transpose(1, 0, 2).reshape(HID, L * H2)
    b1 = np.asarray(inp["conv_b1"], np.float32).reshape(1, L * H2)
    wlin = np.asarray(inp["lin_W"], np.float32)
    blin = np.asarray(inp["lin_b"], np.float32).reshape(1, -1)

    common = dict(
        wnode=np.ascontiguousarray(inp["node_W"], dtype=np.float32),
        bnode=rep(inp["node_b"]),
        wedge=np.ascontiguousarray(wedge_aug.astype(NPBF16)),
        convt=rep(np.asarray(inp["conv_t"], np.float32)),
        w1a=np.ascontiguousarray(np.concatenate([w1, b1], axis=0)),
        g1c=np.ascontiguousarray(np.asarray(inp["conv_g1"], np.float32).T),
        be1c=np.ascontiguousarray(np.asarray(inp["conv_be1"], np.float32).T),
        w2=np.ascontiguousarray(
            np.asarray(inp["conv_W2"], np.float32).transpose(1, 0, 2).reshape(H2, L * HID)),
        b2r=np.ascontiguousarray(np.asarray(inp["conv_b2"], np.float32).reshape(1, -1)),
        lngc=np.ascontiguousarray(np.asarray(inp["ln_g"], np.float32).T),
        lnbc=np.ascontiguousarray(np.asarray(inp["ln_b"], np.float32).T),
        wlina=np.ascontiguousarray(np.concatenate([wlin, blin], axis=0)),
    )
    x = np.asarray(inp["x"], np.float32)
    in_maps = []
    for c in range(R):
        m = dict(common)
        m["xsh"] = np.ascontiguousarray(x[c * NSH:(c + 1) * NSH])
        at = np.concatenate([pre["attr_cols"][c],
                             np.ones((T * 128, 1), np.float32)], axis=1)
        m["attrT"] = np.ascontiguousarray(at.T.astype(NPBF16))
        m["dstrb"] = np.ascontiguousarray(pre["dstr_cols"][c].astype(NPBF16))
        if cfg["gather"] in ("dg", "q4"):
            m["idx16"] = np.ascontiguousarray(pre["idx16"][c])
        else:
            m["srci"] = np.ascontiguousarray(pre["src_cols"][c])
        in_maps.append(m)
    return in_maps




F32 = mybir.dt.float32
BF16 = mybir.dt.bfloat16
I32 = mybir.dt.int32
I16 = mybir.dt.int16
AF = mybir.ActivationFunctionType
OP = mybir.AluOpType
NPBF16 = ml_dtypes.bfloat16


def declare_io(nc, cfg):
    NSH = cfg["NSH"]; NW = cfg["NW"]
    HID = cfg["HID"]; IN = cfg["IN_DIM"]; EA = cfg["EA_DIM"]
    OUT = cfg["OUT_DIM"]; L = cfg["L"]; T = int(sum(cfg["tw"]))
    H2 = 2 * HID
    io = {}

    def inp(name, shape, dt=F32):
        io[name] = nc.dram_tensor(name, shape, dt, kind="ExternalInput")

    inp("xsh", [NSH, IN])
    inp("attrT", [EA, T * 128], BF16)
    inp("dstrb", [128, T], BF16)
    if cfg["gather"] in ("dg", "q4"):
        inp("idx16", [128, T * 8], I16)
    else:
        inp("srci", [128, T], I32)
    inp("wnode", [IN, HID])
    inp("bnode", [128, HID])
    inp("wedge", [EA, HID], BF16)
    inp("convt", [128, L])
    inp("w1a", [HID + 1, L * H2])
    inp("g1c", [H2, L])
    inp("be1c", [H2, L])
    inp("w2", [H2, L * HID])
    inp("b2r", [1, L * HID])
    inp("lngc", [HID, L])
    inp("lnbc", [HID, L])
    inp("wlina", [HID + 1, OUT])
    io["yout"] = nc.dram_tensor("yout", [NW * 128, OUT], F32, kind="ExternalOutput")
    if cfg.get("debug"):
        io["dz0"] = nc.dram_tensor("dz0", [128, NW * HID], F32, kind="ExternalOutput")
        io["da8"] = nc.dram_tensor("da8", [128, cfg["G"] * HID], F32, kind="ExternalOutput")
        io["dsu"] = nc.dram_tensor("dsu", [HID, 256], F32, kind="ExternalOutput")
        io["dhin"] = nc.dram_tensor("dhin", [HID, 128], F32, kind="ExternalOutput")
    return io


def build_graph(tc, ctx, io, cfg):
    nc = tc.nc

    R = cfg["R"]; NSH = cfg["NSH"]; NW = cfg["NW"]; LASTW = cfg["LASTW"]
    NSHP = NW * 128
    HID = cfg["HID"]; IN = cfg["IN_DIM"]; EA = cfg["EA_DIM"]
    OUT = cfg["OUT_DIM"]; L = cfg["L"]; tw = list(cfg["tw"]); G = cfg["G"]
    H2 = 2 * HID
    T = int(sum(tw))
    MSG_EPS = cfg["MSG_EPS"]
    LN_EPS = 1e-5
    NTOT = R * NSHP
    HALF = NTOT // 2

    zin = [nc.dram_tensor(f"zin{l}", [NSHP, HID], F32) for l in range(L)]
    zfull = [nc.dram_tensor(f"zfull{l}", [NTOT, HID], F32, addr_space="Shared")
             for l in range(L)]

    const = ctx.enter_context(tc.tile_pool(name="const", bufs=1))
    sb = ctx.enter_context(tc.tile_pool(name="sbp", bufs=3))
    gpool = ctx.enter_context(tc.tile_pool(name="gpool", bufs=3))
    npool = ctx.enter_context(tc.tile_pool(name="npool", bufs=2))
    psum = ctx.enter_context(tc.tile_pool(name="psum", bufs=3, space="PSUM"))
    eapool = ctx.enter_context(tc.tile_pool(name="eapool", bufs=2, space="PSUM"))
    supool = ctx.enter_context(tc.tile_pool(name="supool", bufs=2, space="PSUM"))

    # ---- constants ----
    ident = const.tile([128, 128], F32)
    make_identity(nc, ident[:])
    iota_i = const.tile([128, 128], I32)
    nc.gpsimd.iota(iota_i[:], pattern=[[1, 128]], base=0, channel_multiplier=0)
    iota_b = const.tile([128, 128], BF16)
    nc.vector.tensor_copy(iota_b[:], iota_i[:])
    eps_ln = const.tile([128, 1], F32)
    nc.vector.memset(eps_ln[:], LN_EPS)
    iota_p = const.tile([128, 1], I32)
    nc.gpsimd.iota(iota_p[:], pattern=[[1, 1]], base=0, channel_multiplier=1)
    rowmask = const.tile([128, 1], F32)
    nc.vector.tensor_scalar(rowmask[:], iota_p[:], float(LASTW), None, op0=OP.is_lt)
    ones1 = const.tile([1, 128], F32)
    nc.vector.memset(ones1[:], 1.0)

    names = ["wnode", "bnode", "wedge", "convt", "w1a", "g1c", "be1c",
             "w2", "b2r", "lngc", "lnbc", "wlina", "dstrb"]
    names += ["idx16"] if cfg["gather"] in ("dg", "q4") else ["srci"] if cfg["gather"] == "ind" else []
    S = {}
    for nm in names:
        t = io[nm]
        S[nm] = const.tile(list(t.shape), t.dtype, name=f"{nm}_sb")
        nc.sync.dma_start(S[nm][:], t[:])

    h_sb = const.tile([128, NW * HID], F32)     # residual h, node-major
    z_sb = const.tile([128, NW * HID], F32)     # conv input z, node-major
    zT_sb = const.tile([HID, NW * 128], F32)    # z transposed, feature-major
    yout_sb = const.tile([128, NW * OUT], F32)

    toff = np.concatenate([[0], np.cumsum(tw)]).astype(int)
    t2w = np.repeat(np.arange(NW), tw).astype(int)

    def wsl(tl, w, d):
        return tl[:, w * d:(w + 1) * d]

    RSQRT_MAGIC = 0x5F3759DF - (1 << 22)  # rsqrt seed magic for hv = v/2

    def ln_cen(src_ap, D, ctag):
        """Center+scale rows of src: cen = (src - mean)*rstd, [128, D]."""
        stats = npool.tile([128, 6], F32, tag="stats")
        nc.vector.bn_stats(stats[:], src_ap)
        mv = npool.tile([128, 2], F32, tag="mv")
        nc.vector.bn_aggr(mv[:], stats[:])
        rstd = npool.tile([128, 1], F32, tag="rstd")
        if cfg.get("rstd") == "quake":
            # rstd = (var+eps)^-0.5 on DVE (quake seed + 1 Newton step)
            hv = npool.tile([128, 1], F32, tag="hv")
            nc.vector.tensor_scalar(hv[:], mv[:, 1:2], LN_EPS, 0.5,
                                    op0=OP.add, op1=OP.mult)
            sd = npool.tile([128, 1], I32, tag="sd")
            nc.vector.tensor_scalar(sd[:], hv[:].bitcast(I32), 1, -1,
                                    op0=OP.logical_shift_right,
                                    op1=OP.bitwise_xor)
            nc.vector.tensor_scalar(sd[:], sd[:], RSQRT_MAGIC + 1, None,
                                    op0=OP.add)
            y0 = sd[:].bitcast(F32)
            nt = npool.tile([128, 1], F32, tag="nt")
            nc.vector.tensor_tensor(nt[:], y0, y0, op=OP.mult)
            nc.vector.tensor_tensor(nt[:], nt[:], hv[:], op=OP.mult)
            nc.vector.tensor_scalar(nt[:], nt[:], -1.0, 1.5,
                                    op0=OP.mult, op1=OP.add)
            nc.vector.tensor_tensor(rstd[:], y0, nt[:], op=OP.mult)
        else:
            lnv = npool.tile([128, 1], F32, tag="lnv")
            nc.scalar.activation(lnv[:], mv[:, 1:2], AF.Ln, bias=eps_ln[:],
                                 scale=1.0)
            nc.scalar.activation(rstd[:], lnv[:], AF.Exp, bias=0.0, scale=-0.5)
        cen = npool.tile([128, D], F32, tag=ctag)
        nc.vector.tensor_scalar(cen[:], src_ap, mv[:, 0:1], rstd[:],
                                op0=OP.subtract, op1=OP.mult)
        return cen

    def pe_transpose(dst_sb_ap, src_sb_ap, use_scalar_copy=False):
        """dst = src.T via PE; dst partitions = src free size."""
        pfree = src_sb_ap.shape[0]
        ps = psum.tile([128, 128], F32, tag="mm")
        tview = ps[:src_sb_ap.shape[1], :pfree]
        nc.tensor.transpose(out=tview, in_=src_sb_ap, identity=ident[:])
        if use_scalar_copy:
            nc.scalar.copy(dst_sb_ap, tview)
        else:
            nc.vector.tensor_copy(dst_sb_ap, tview)

    # ---- setup: h0 = x @ Wn + bn; z0 = h0 ----
    for w in range(NW):
        rows = 128 if w < NW - 1 else LASTW
        xt = sb.tile([128, IN], F32, tag="xt")
        if rows < 128:
            nc.vector.memset(xt[:], 0.0)
        nc.sync.dma_start(xt[:rows, :], io["xsh"][w * 128:w * 128 + rows, :])
        xT_ps = psum.tile([128, 128], F32, tag="mm")
        nc.tensor.transpose(out=xT_ps[:IN, :], in_=xt[:], identity=ident[:])
        xT = sb.tile([IN, 128], F32, tag="xT")
        nc.scalar.copy(xT[:], xT_ps[:IN, :])
        h_ps = psum.tile([128, 128], F32, tag="mm")
        nc.tensor.matmul(h_ps[:, :HID], lhsT=xT[:], rhs=S["wnode"][:],
                         start=True, stop=True)
        nc.vector.tensor_tensor(wsl(h_sb, w, HID), h_ps[:, :HID], S["bnode"][:],
                                op=OP.add)
        if w == NW - 1 and LASTW < 128:
            nc.vector.tensor_scalar(wsl(z_sb, w, HID), wsl(h_sb, w, HID),
                                    rowmask[:], None, op0=OP.mult)
        else:
            nc.vector.tensor_copy(wsl(z_sb, w, HID), wsl(h_sb, w, HID))
        pe_transpose(wsl(zT_sb, w, 128), wsl(z_sb, w, HID), use_scalar_copy=True)
    nc.sync.dma_start(
        zin[0][:].rearrange("(p w) h -> p (w h)", w=NW), z_sb[:])
    if cfg.get("debug"):
        nc.sync.dma_start(io["dz0"][:], z_sb[:])

    def node_phase(li, w, su):
        # su: [HID, 2*128] psum: S^T cols 0:128, U^T cols 128:256
        sT = npool.tile([HID, 128], F32, tag="sT")
        nc.vector.tensor_scalar(sT[:], su[:HID, 0:128], 1e-16, None, op0=OP.add)
        rT = npool.tile([HID, 128], F32, tag="rT")
        nc.vector.reciprocal_approx_fast(rT[:], sT[:])
        hinT = npool.tile([HID + 1, 128], F32, tag="hinT")
        nc.gpsimd.memset(hinT[HID:HID + 1, :], 1.0)
        nc.vector.tensor_tensor(hinT[:HID, :], su[:HID, 128:256], rT[:],
                                op=OP.mult)
        nc.vector.tensor_tensor(hinT[:HID, :], hinT[:HID, :],
                                wsl(zT_sb, w, 128), op=OP.add)
        mm1 = psum.tile([128, 128], F32, tag="mm")
        nc.tensor.matmul(mm1[:, :H2], lhsT=hinT[:],
                         rhs=S["w1a"][:, li * H2:(li + 1) * H2],
                         start=True, stop=True)
        cen = ln_cen(mm1[:, :H2], H2, "cen")
        cT = psum.tile([128, 128], F32, tag="mm")
        nc.tensor.transpose(out=cT[:H2, :], in_=cen[:], identity=ident[:])
        # y1T = relu(g1*cenT + be1): per-partition affine+relu on Act
        y1T = npool.tile([H2, 128], F32, tag="y1T")
        nc.scalar.activation(y1T[:], cT[:H2, :], AF.Relu,
                             bias=S["be1c"][:, li:li + 1],
                             scale=S["g1c"][:, li:li + 1])
        mm2 = psum.tile([128, 128], F32, tag="mm")
        nc.tensor.matmul(mm2[:, :HID], lhsT=y1T[:],
                         rhs=S["w2"][:, li * HID:(li + 1) * HID],
                         start=True, stop=False)
        nc.tensor.matmul(mm2[:, :HID], lhsT=ones1[:],
                         rhs=S["b2r"][:, li * HID:(li + 1) * HID],
                         start=False, stop=True)
        hw = wsl(h_sb, w, HID)
        if li == 0:
            nc.scalar.copy(hw, mm2[:, :HID])
        else:
            nc.vector.tensor_tensor(hw, hw, mm2[:, :HID], op=OP.add)
        if li < L - 1:
            cen2 = ln_cen(hw, HID, "cen2")
            c2T = psum.tile([128, 128], F32, tag="mm")
            nc.tensor.transpose(out=c2T[:HID, :], in_=cen2[:], identity=ident[:])
            nc.scalar.activation(wsl(zT_sb, w, 128), c2T[:HID, :], AF.Relu,
                                 bias=S["lnbc"][:, li + 1:li + 2],
                                 scale=S["lngc"][:, li + 1:li + 2])
            zb = psum.tile([128, 128], F32, tag="mm")
            nc.tensor.transpose(out=zb[:, :HID], in_=wsl(zT_sb, w, 128),
                                identity=ident[:HID, :HID])
            if w == NW - 1 and LASTW < 128:
                nc.vector.tensor_scalar(wsl(z_sb, w, HID), zb[:, :HID],
                                        rowmask[:], None, op0=OP.mult)
            else:
                nc.scalar.copy(wsl(z_sb, w, HID), zb[:, :HID])
        else:
            cen3 = ln_cen(hw, HID, "cen2")
            c3T = psum.tile([128, 128], F32, tag="mm")
            nc.tensor.transpose(out=c3T[:HID, :], in_=cen3[:], identity=ident[:])
            zfT = npool.tile([HID + 1, 128], F32, tag="zfT")
            nc.gpsimd.memset(zfT[HID:HID + 1, :], 1.0)
            nc.scalar.activation(zfT[:HID, :], c3T[:HID, :], AF.Relu,
                                 bias=S["lnbc"][:, 0:1], scale=S["lngc"][:, 0:1])
            mmo = psum.tile([128, 128], F32, tag="mm")
            nc.tensor.matmul(mmo[:, :OUT], lhsT=zfT[:], rhs=S["wlina"][:],
                             start=True, stop=True)
            if w == NW - 1 and LASTW < 128:
                nc.vector.tensor_scalar(wsl(yout_sb, w, OUT), mmo[:, :OUT],
                                        rowmask[:], None, op0=OP.mult)
            else:
                nc.scalar.copy(wsl(yout_sb, w, OUT), mmo[:, :OUT])

    GRUN = cfg.get("GRUN", 16)

    def edge_phase(li, rep=0):
        zf = zfull[li]
        probe = sb.tile([1, HID], F32, tag="probe")
        nc.gpsimd.dma_start(probe[:], zf[:1, :])
        twh = cfg["twh"]
        # gather runs: consecutive tiles of one (window, half), <= GRUN each
        runs = []
        t = 0
        for w in range(NW):
            for h in (0, 1):
                nt = int(twh[w][h])
                while nt > 0:
                    gn = min(GRUN, nt)
                    runs.append((t, gn, h))
                    t += gn
                    nt -= gn
        assert t == T
        su_tiles = {}
        for (t0, gn, h) in runs:
            gbuf = gpool.tile([128, GRUN * HID], F32, tag="gbuf")
            gb3 = gbuf[:, :gn * HID].rearrange("p (c h) -> p c h", h=HID)
            src_half = zf[0:HALF, :] if h == 0 else zf[HALF:NTOT, :]
            if cfg.get("gather_seq"):
                base = (t0 * 128) % (HALF - GRUN * 128)
                nc.sync.dma_start(
                    gb3,
                    zf[base:base + gn * 128, :].rearrange("(c p) h -> p c h", p=128))
            else:
                qn = (t0 // GRUN) % 4 if cfg.get("qspread") else 0
                nc.gpsimd.dma_gather(
                    out_ap=gb3, in_ap=src_half,
                    idxs_ap=S["idx16"][:, t0 * 8:(t0 + gn) * 8],
                    num_idxs=gn * 128, num_idxs_reg=gn * 128, elem_size=HID,
                    queue_num=qn)
            at = sb.tile([EA, GRUN * 128], BF16, tag="at")
            nc.sync.dma_start(at[:, :gn * 128],
                              io["attrT"][:, t0 * 128:(t0 + gn) * 128])
            # PSUM-sized chunks of G tiles within the run
            for c0 in range(0, gn, G):
                cn = min(G, gn - c0)
                ea_ps = eapool.tile([128, G * HID], F32, tag="eaps")
                for k in range(cn):
                    nc.tensor.matmul(ea_ps[:, k * HID:(k + 1) * HID],
                                     lhsT=at[:, (c0 + k) * 128:(c0 + k + 1) * 128],
                                     rhs=S["wedge"][:], start=True, stop=True)
                a8 = sb.tile([128, G * HID], BF16, tag="a8")
                nc.vector.tensor_tensor(a8[:, :cn * HID],
                                        gbuf[:, c0 * HID:(c0 + cn) * HID],
                                        ea_ps[:, :cn * HID], op=OP.add)
                nc.vector.tensor_scalar(a8[:, :cn * HID], a8[:, :cn * HID],
                                        0.0, MSG_EPS, op0=OP.max, op1=OP.add)
                p8 = sb.tile([128, G * HID], BF16, tag="p8")
                nc.scalar.activation(p8[:, :cn * HID], a8[:, :cn * HID],
                                     AF.Exp, bias=0.0,
                                     scale=S["convt"][:, li:li + 1])
                q8 = sb.tile([128, G * HID], BF16, tag="q8")
                nc.vector.tensor_tensor(q8[:, :cn * HID], a8[:, :cn * HID],
                                        p8[:, :cn * HID], op=OP.mult)
                # batched one-hot: ohb[:, k*128+j] = (iota[j] == dstr[:, t0+c0+k])
                ohb = sb.tile([128, G * 128], BF16, tag="ohb")
                ib = iota_b[:]
                in0v = bass.AP(ib.tensor, ib.offset,
                               [ib.ap[0], [0, cn], ib.ap[1]])
                dsl = S["dstrb"][:, t0 + c0:t0 + c0 + cn]
                in1v = bass.AP(dsl.tensor, dsl.offset,
                               [dsl.ap[0], dsl.ap[1], [0, 128]])
                nc.vector.tensor_tensor(
                    ohb[:, :cn * 128].rearrange("p (c n) -> p c n", n=128),
                    in0v, in1v, op=OP.is_equal)
                for k in range(cn):
                    t = t0 + c0 + k
                    oh = ohb[:, k * 128:(k + 1) * 128]
                    w = int(t2w[t])
                    if w not in su_tiles:
                        su_tiles[w] = supool.tile([HID, 256], F32, tag="su",
                                                  name=f"su{rep}_{li}_{w}")
                    first = (t == toff[w])
                    last = (t == toff[w + 1] - 1)
                    nc.tensor.matmul(su_tiles[w][:, 0:128],
                                     lhsT=p8[:, k * HID:(k + 1) * HID], rhs=oh,
                                     start=first, stop=False)
                    nc.tensor.matmul(su_tiles[w][:, 128:256],
                                     lhsT=q8[:, k * HID:(k + 1) * HID], rhs=oh,
                                     start=False, stop=last)
                    if last:
                        node_phase(li, w, su_tiles.pop(w))

    for rep in range(cfg.get("rep", 1)):
        for li in range(L):
            with nc.named_scope(f"AG{li}"):
                if cfg.get("sim_local_ag"):
                    # single-core timing sim stand-in for the AllGather
                    nc.sync.dma_start(zfull[li][0:NSHP, :], zin[li][:])
                else:
                    nc.gpsimd.collective_compute(
                        "AllGather", OP.bypass, replica_groups=[list(range(R))],
                        ins=[zin[li][:]], outs=[zfull[li][:]])
            with nc.named_scope(f"edge{li}"):
                edge_phase(li, rep)
            if li < L - 1:
                nc.sync.dma_start(
                    zin[li + 1][:].rearrange("(p w) h -> p (w h)", w=NW), z_sb[:])
        if rep < cfg.get("rep", 1) - 1:
            # re-seed z/zT/h for the next repetition (timing only)
            nc.sync.dma_start(
                zin[0][:].rearrange("(p w) h -> p (w h)", w=NW), z_sb[:])

    nc.sync.dma_start(
        io["yout"][:].rearrange("(p w) o -> p (w o)", w=NW), yout_sb[:])


def build_spmd(nc, n_cores):
    install_neuronx_cc_hook()
    partition_name = nc.partition_id_tensor.name if nc.partition_id_tensor else None
    in_names, out_names, out_avals, zero_outs = [], [], [], []
    for alloc in nc.m.functions[0].allocations:
        if not isinstance(alloc, mybir.MemoryLocationSet):
            continue
        name = alloc.memorylocations[0].name
        if alloc.kind == "ExternalInput":
            if name != partition_name:
                in_names.append(name)
        elif alloc.kind == "ExternalOutput":
            out_avals.append(jax.core.ShapedArray(
                tuple(alloc.tensor_shape), mybir.dt.np(alloc.dtype)))
            out_names.append(name)
            zero_outs.append(np.zeros(alloc.tensor_shape, mybir.dt.np(alloc.dtype)))

    n_params = len(in_names)
    n_outs = len(out_avals)
    all_in_names = list(in_names) + list(out_names)
    if partition_name is not None:
        all_in_names.append(partition_name)

    def _body(*args):
        operands = list(args)
        if partition_name is not None:
            operands.append(partition_id_tensor())
        outs = _bass_exec_p.bind(
            *operands,
            out_avals=tuple(out_avals),
            in_names=tuple(all_in_names),
            out_names=tuple(out_names),
            lowering_input_output_aliases=(),
            sim_require_finite=True,
            sim_require_nnan=True,
            nc=nc,
        )
        return tuple(outs)

    devices = jax.devices()[:n_cores]
    mesh = Mesh(np.asarray(devices), ("core",))
    in_specs = (PartitionSpec("core"),) * (n_params + n_outs)
    out_specs = (PartitionSpec("core"),) * len(out_names)
    sharded = jax.jit(
        shard_map(_body, mesh=mesh, in_specs=in_specs, out_specs=out_specs,
                  check_rep=False),
        keep_unused=True,
    )
    return dict(fn=sharded, in_names=in_names, out_names=out_names,
                out_avals=out_avals, zero_outs=zero_outs, mesh=mesh,
                n_cores=n_cores)


def run_spmd(rt, in_maps, n_timing_iters=0):
    """Returns (results_per_core, times_s list)."""
    n_cores = rt["n_cores"]
    mesh = rt["mesh"]
    sh = jax.sharding.NamedSharding(mesh, PartitionSpec("core"))
    concat_in = [
        np.concatenate([np.asarray(in_maps[c][name]) for c in range(n_cores)], axis=0)
        for name in rt["in_names"]
    ]
    concat_zeros = [
        np.zeros((n_cores * z.shape[0], *z.shape[1:]), z.dtype)
        for z in rt["zero_outs"]
    ]
    dev_in = [jax.device_put(a, sh) for a in concat_in]
    dev_zeros = [jax.device_put(a, sh) for a in concat_zeros]
    out = rt["fn"](*dev_in, *dev_zeros)
    jax.block_until_ready(out)
    times = []
    for _ in range(n_timing_iters):
        t0 = time.perf_counter()
        out2 = rt["fn"](*dev_in, *dev_zeros)
        jax.block_until_ready(out2)
        times.append(time.perf_counter() - t0)
    results = [
        {
            name: np.asarray(out[i]).reshape(n_cores, *rt["out_avals"][i].shape)[c]
            for i, name in enumerate(rt["out_names"])
        }
        for c in range(n_cores)
    ]
    return results, times


_state = {}


def build_nc(cfg):
    bacc_cls = BaccMonoActTable if cfg.get("mono") else bacc.Bacc
    nc = bacc_cls(None, target_bir_lowering=False, debug=False,
                  num_devices=cfg["R"],
                  dynamic_dma_scratch_size=cfg.get("ddss", 65536),
                  num_swdge_queues=4 if cfg.get("qspread") else 1)
    with tile.TileContext(nc) as tc:
        with ExitStack() as ctx:
            io = declare_io(nc, cfg)
            build_graph(tc, ctx, io, cfg)
    nc.finalize()
    return nc


def kernel(**inputs):
    import os
    cfg = make_cfg(G=8, gather="dg")
    cfg["GRUN"] = int(os.environ.get("K_GRUN", "8"))
    cfg["ddss"] = int(os.environ.get("K_DDSS", "16384"))
    cfg["rstd"] = os.environ.get("K_RSTD", "lnexp")
    inp = {k: np.asarray(v) for k, v in inputs.items()}
    pre = preprocess_edges(cfg, inp["edge_index"], inp["edge_attr"])
    cfg["tw"] = pre["tw"]
    cfg["twh"] = pre["twh"]
    in_maps = prep_inputs(cfg, inp, pre)

    nc = build_nc(cfg)

    rt = build_spmd(nc, cfg["R"])
    res, _ = run_spmd(rt, in_maps, 0)

    NSH, NW = cfg["NSH"], cfg["NW"]
    n = np.arange(NSH)
    rows = (n % 128) * NW + n // 128
    out = np.concatenate([res[c]["yout"][rows] for c in range(cfg["R"])], axis=0)
    _state.update(rt=rt, in_maps=in_maps, cfg=cfg)
    return out.astype(np.float32)


def measure_exec_ns(iters=12):
    """Wall-clock kernel estimate: min(full) - min(trivial baseline), ns."""
    rt, in_maps, cfg = _state["rt"], _state["in_maps"], _state["cfg"]
    nc0 = bacc.Bacc(None, target_bir_lowering=False, debug=False,
                    num_devices=cfg["R"])
    bx = nc0.dram_tensor("bx", [128, 64], mybir.dt.float32, kind="ExternalInput")
    by = nc0.dram_tensor("by", [128, 64], mybir.dt.float32, kind="ExternalOutput")
    with tile.TileContext(nc0) as tc0:
        with tc0.tile_pool(name="sb", bufs=2) as sb0:
            t_ = sb0.tile([128, 64], mybir.dt.float32)
            nc0.sync.dma_start(t_[:], bx[:])
            nc0.sync.dma_start(by[:], t_[:])
    nc0.finalize()
    rt0 = build_spmd(nc0, cfg["R"])
    bmap = [{"bx": np.zeros((128, 64), np.float32)} for _ in range(cfg["R"])]
    run_spmd(rt0, bmap, 0)
    times, btimes = [], []
    for _ in range(iters):
        _, ts = run_spmd(rt, in_maps, 1)
        times.extend(ts)
        _, bs = run_spmd(rt0, bmap, 1)
        btimes.extend(bs)
    return (min(times) - min(btimes)) * 1e9

